# revision 19
# baseline (speedup 1.0000x reference)
"""TRN2 Bass kernel for GQA attention (nn_Attention_19533511262498).

Tensor-parallel over heads across 8 NeuronCores: core c owns q-heads
[4c, 4c+4) and kv-head c (wq/wk/wv sharded on the head dim, wo on its
input dim). Each core computes a partial [S, DIM] output; the host sums
the 8 partials.

All matmuls run in float32r (tf32-like, ~11 mantissa bits, full PE rate
at moving-dim >= 256). Everything is kept in transposed ([feature, seq])
layouts; V is transposed on-chip via PE identity-matmuls. Softmax skips
the max-subtraction (scores are bounded by construction) and the 1/l
normalization is applied to the attention output right before the wo
projection, broadcast across partitions with a rank-1 ones matmul and
pipelined one head behind the score loop so the PE never stalls on it.

RoPE trick: wq/wk rows are permuted on the host (even dims first, odd
dims second within each head) so the rotation becomes pure elementwise
ops plus a half-tile partition swap done by SBUF-to-SBUF DMA; the
permutation cancels in q.k dot products.

Inputs are pre-tiled on the host into [128, chunk, free] layouts so
every HBM DMA moves long contiguous per-partition runs.
"""

import numpy as np

import concourse.bacc as bacc
import concourse.tile as tile
from concourse import mybir
from concourse.bass import ts, ds
from concourse import bass_isa
from concourse.bass_utils import run_bass_kernel_spmd

F32 = mybir.dt.float32
F32R = mybir.dt.float32r

# problem geometry (hardcoded per contest rules)
S = 2048
DIM = 4096
HD = 128
N_HEADS = 32
N_KV = 8
NCORES = 8
HPC = N_HEADS // NCORES       # 4 q heads per core
FEAT = HPC * HD               # 512 per-core attention feature width

SBW = 256                     # QKV projection s-block width
NSB = S // SBW                # 8
KCH = DIM // 128              # 32 contraction chunks
QBW = 512                     # attention q-block width
NQB = S // QBW                # 4
NSC = S // 128                # 16 kv chunks
OBW = 512                     # output-dim block width
NOB = DIM // OBW              # 8
NRT = HPC + 1                 # rope targets per s-block (4 q heads + k)

_CACHE = {}


def _build():
    nc = bacc.Bacc("TRN2", target_bir_lowering=False, debug=False,
                   num_devices=NCORES)

    xT = nc.dram_tensor("xT", [NSB, 128, KCH, SBW], F32R, kind="ExternalInput").ap()
    wqT = nc.dram_tensor("wqT", [HPC, 128, KCH, HD], F32R, kind="ExternalInput").ap()
    wkT = nc.dram_tensor("wkT", [128, KCH, HD], F32R, kind="ExternalInput").ap()
    wvT = nc.dram_tensor("wvT", [128, KCH, HD], F32R, kind="ExternalInput").ap()
    woT = nc.dram_tensor("woT", [HPC, 128, DIM], F32R, kind="ExternalInput").ap()
    cos2 = nc.dram_tensor("cos2", [128, S], F32, kind="ExternalInput").ap()
    sin2 = nc.dram_tensor("sin2", [128, S], F32, kind="ExternalInput").ap()
    gmask = nc.dram_tensor("gmask", [128, 896], F32, kind="ExternalInput").ap()
    onesc = nc.dram_tensor("onesc", [128, 1], F32R, kind="ExternalInput").ap()
    onesr = nc.dram_tensor("onesr", [1, 128], F32R, kind="ExternalInput").ap()
    sgn = nc.dram_tensor("sgn", [128, 1], F32, kind="ExternalInput").ap()
    ident = nc.dram_tensor("ident", [128, 128], F32, kind="ExternalInput").ap()
    identn = nc.dram_tensor("identn", [128, 128], F32R, kind="ExternalInput").ap()
    hmask = nc.dram_tensor("hmask", [128, 896], F32R, kind="ExternalInput").ap()
    out_d = nc.dram_tensor("out", [S, DIM], F32, kind="ExternalOutput").ap()

    with tile.TileContext(nc) as tc:
        with (
            tc.tile_pool(name="dram", bufs=1, space="DRAM") as dpool,
            tc.tile_pool(name="res", bufs=1) as res,
        ):
            qt_ds = [dpool.tile([HPC, 128, SBW], F32R, tag=f"qt{sb}", name=f"qt_d{sb}")
                     for sb in range(NSB)]

            kt_t = res.tile([128, S], F32R, tag="kt")
            vt_t = res.tile([128, S], F32, tag="vt")
            onesc_t = res.tile([128, 1], F32R, tag="onesc")
            onesr_t = res.tile([1, 128], F32R, tag="onesr")
            sgn_t = res.tile([128, 1], F32, tag="sgn")
            nc.sync.dma_start(out=onesc_t, in_=onesc)
            nc.sync.dma_start(out=onesr_t, in_=onesr)
            nc.sync.dma_start(out=sgn_t, in_=sgn)

            # ---------------- Phase 1: QKV projections + RoPE ----------------
            with (
                tc.tile_pool(name="wq", bufs=1) as wqp,
                tc.tile_pool(name="wkv", bufs=1) as wkvp,
                tc.tile_pool(name="xt", bufs=4) as xtp,
                tc.tile_pool(name="trig", bufs=2) as trigp,
                tc.tile_pool(name="rope", bufs=6) as ropep,
                tc.tile_pool(name="stage1", bufs=2) as st1p,
                tc.tile_pool(name="qkvps", bufs=3, space="PSUM") as qkvps,
            ):
                # sync-FIFO order matters: wk/wv + first x block before the
                # 8MB of wq so the k/v matmuls can start at ~15us.
                wk_t = wkvp.tile([128, KCH, HD], F32R, tag="wk")
                nc.sync.dma_start(out=wk_t, in_=wkT)
                KH = KCH // 2

                def load_xt(sb):
                    a = xtp.tile([128, KH, SBW], F32R, tag="xt", name=f"xta{sb}")
                    nc.sync.dma_start(out=a, in_=xT[sb, :, 0:KH])
                    b = xtp.tile([128, KH, SBW], F32R, tag="xt", name=f"xtb{sb}")
                    nc.sync.dma_start(out=b, in_=xT[sb, :, KH:KCH])
                    return (a, b)

                xt_tiles = {0: load_xt(0)}
                wq_hs = [wqp.tile([128, KCH, HD], F32R, tag=f"wq{h}", name=f"wq{h}")
                         for h in range(HPC)]
                nc.sync.dma_start(out=wq_hs[0], in_=wqT[0])
                wv_t = wkvp.tile([128, KCH, HD], F32R, tag="wv")
                nc.sync.dma_start(out=wv_t, in_=wvT)
                for h in range(1, HPC):
                    nc.sync.dma_start(out=wq_hs[h], in_=wqT[h])

                for sb in range(NSB):
                    if sb not in xt_tiles:
                        xt_tiles[sb] = load_xt(sb)
                    xt_a, xt_b = xt_tiles[sb]
                    c_sl = trigp.tile([128, SBW], F32, tag="cos")
                    nc.scalar.dma_start(out=c_sl, in_=cos2[:, ts(sb, SBW)])
                    s_sl = trigp.tile([128, SBW], F32, tag="sin")
                    nc.scalar.dma_start(out=s_sl, in_=sin2[:, ts(sb, SBW)])
                    qst = st1p.tile([128, HPC, SBW], F32R, tag="qst")
                    # k, v first so matmuls start before the larger wq loads land
                    for ob in [HPC, 0, HPC + 1, 1, 2, 3]:
                        ps = qkvps.tile([128, SBW], F32, tag="ps")
                        for k in range(KCH):
                            if ob < HPC:
                                lhs = wq_hs[ob][:, k, :]
                            elif ob == HPC:
                                lhs = wk_t[:, k, :]
                            else:
                                lhs = wv_t[:, k, :]
                            rhs = xt_a[:, k, :] if k < KH else xt_b[:, k - KH, :]
                            nc.tensor.matmul(ps, lhs, rhs,
                                             start=(k == 0), stop=(k == KCH - 1))
                        if ob <= HPC:
                            # RoPE: rot = (swap_halves(x*sin) * sgn) + x*cos
                            m1 = ropep.tile([128, SBW], F32, tag="m1")
                            m2 = ropep.tile([128, SBW], F32, tag="m2")
                            w = ropep.tile([128, SBW], F32, tag="w")
                            nc.vector.tensor_mul(m1, ps, c_sl)
                            nc.vector.tensor_mul(m2, ps, s_sl)
                            nc.sync.dma_start(out=w[0:64], in_=m2[64:128])
                            nc.sync.dma_start(out=w[64:128], in_=m2[0:64])
                            dst = qst[:, ob] if ob < HPC else kt_t[:, ts(sb, SBW)]
                            nc.vector.scalar_tensor_tensor(
                                dst, w, sgn_t, m1,
                                op0=mybir.AluOpType.mult, op1=mybir.AluOpType.add)
                        else:
                            nc.vector.tensor_copy(vt_t[:, ts(sb, SBW)], ps)
                    nc.scalar.dma_start(
                        out=qt_ds[sb].rearrange("h p s -> p h s"), in_=qst)

            # ---------------- Phase 2: attention + output projection --------
            with (
                tc.tile_pool(name="wo", bufs=1) as wop,
                tc.tile_pool(name="vres", bufs=1) as vresp,
                tc.tile_pool(name="consts2", bufs=1) as c2p,
                tc.tile_pool(name="qt", bufs=4) as qtp,
                tc.tile_pool(name="exp", bufs=6) as expp,
                tc.tile_pool(name="outT", bufs=8) as outTp,
                tc.tile_pool(name="rc", bufs=4) as rcp,
                tc.tile_pool(name="small", bufs=4) as smallp,
                tc.tile_pool(name="stage2", bufs=3) as st2p,
                tc.tile_pool(name="sum", bufs=3) as sump,
                tc.tile_pool(name="bc", bufs=4) as bcp,
                tc.tile_pool(name="scps", bufs=2, space="PSUM") as scps,
                tc.tile_pool(name="pvps", bufs=3, space="PSUM") as pvps,
                tc.tile_pool(name="prps", bufs=3, space="PSUM") as prps,
            ):
                id_t = c2p.tile([128, 128], F32, tag="id")
                nc.sync.dma_start(out=id_t, in_=ident)
                idn_t = c2p.tile([128, 128], F32R, tag="idn")
                nc.sync.dma_start(out=idn_t, in_=identn)
                hm_t = c2p.tile([128, 896], F32R, tag="hm")
                nc.sync.dma_start(out=hm_t, in_=hmask)
                # wo on the scalar ring so phase-2 qt loads aren't queued
                # behind 8MB on the sync FIFO
                wo_hs = []
                for h in range(HPC):
                    wo_h = wop.tile([128, DIM], F32R, tag=f"wo{h}")
                    nc.scalar.dma_start(out=wo_h, in_=woT[h])
                    wo_hs.append(wo_h)
                # on-chip V transpose: v[s, hd] tiles from vt[hd, s]
                v_t = vresp.tile([128, NSC, HD], F32R, tag="v")
                for sc in range(NSC):
                    tr_ps = scps.tile([128, HD], F32, tag="sc")
                    nc.tensor.transpose(tr_ps, vt_t[:, ts(sc, 128)], id_t)
                    nc.vector.tensor_copy(v_t[:, sc, :], tr_ps)

                # Normalizers run 2 heads behind the score loop (chunked so
                # the first 128-q slice is ready ~1.5us after the sums); the
                # whole projection runs one q-block behind the attention.
                pend = []  # list of (pv_ps, sum_t, outT_t)

                def emit_normalize():
                    pv_ps, sum_t, outT_t = pend.pop(0)
                    for cs in range(QBW // 128):
                        c = ts(cs, 128)
                        bc_c = bcp.tile([128, 128], F32, tag="bc")
                        nc.gpsimd.partition_all_reduce(
                            bc_c, sum_t[:, c], channels=128,
                            reduce_op=bass_isa.ReduceOp.add)
                        rc_c = rcp.tile([128, 128], F32, tag="rc")
                        nc.vector.reciprocal_approx_fast(out=rc_c, in_=bc_c)
                        nc.vector.tensor_mul(outT_t[:, c], pv_ps[:, c], rc_c)

                def emit_proj_slot(pqb, tiles, qs):
                    o_st = st2p.tile([128, DIM], F32, tag="ost")
                    for ob in range(NOB):
                        p_ps = prps.tile([128, OBW], F32, tag="pr")
                        for h in range(HPC):
                            nc.tensor.matmul(p_ps,
                                             tiles[h][:, ts(qs, 128)],
                                             wo_hs[h][:, ts(ob, OBW)],
                                             start=(h == 0), stop=(h == HPC - 1))
                        nc.vector.tensor_copy(o_st[:, ts(ob, OBW)], p_ps)
                    nc.scalar.dma_start(
                        out=out_d[ds(pqb * QBW + qs * 128, 128), :], in_=o_st)

                prev_proj = None  # (qb, outT_tiles)
                for qb in range(NQB):
                    outT_tiles = []
                    for h in range(HPC):
                        qt_t = qtp.tile([128, 2, SBW], F32R, tag="qt")
                        nc.sync.dma_start(out=qt_t[:, 0, :],
                                          in_=qt_ds[2 * qb][h].rearrange("p s -> p s"))
                        nc.sync.dma_start(out=qt_t[:, 1, :],
                                          in_=qt_ds[2 * qb + 1][h])
                        qt_v = qt_t.rearrange("p b s -> p (b s)")
                        pv_ps = pvps.tile([128, QBW], F32, tag="pv")
                        sum_t = sump.tile([128, QBW], F32, tag="sum")
                        nsc = 4 * (qb + 1)
                        for sc in range(nsc):
                            s_ps = scps.tile([128, QBW], F32, tag="sc")
                            diag = sc >= 4 * qb
                            nc.tensor.matmul(s_ps, kt_t[:, ts(sc, 128)], qt_v,
                                             start=True, stop=not diag)
                            if diag:
                                # causal mask: accumulate -1e5 * step pattern
                                t = sc - 4 * qb
                                nc.tensor.matmul(
                                    s_ps, idn_t, hm_t[:, ds(384 - 128 * t, QBW)],
                                    start=False, stop=True)
                            e_t = expp.tile([128, QBW], F32R, tag="exp")
                            nc.scalar.activation(e_t, s_ps,
                                                 mybir.ActivationFunctionType.Exp)
                            e32 = e_t.bitcast(F32)
                            if sc == 0:
                                nc.vector.tensor_copy(sum_t, e32)
                            else:
                                nc.vector.tensor_add(sum_t, sum_t, e32)
                            nc.tensor.matmul(pv_ps, v_t[:, sc, :], e_t,
                                             start=(sc == 0), stop=(sc == nsc - 1))
                        outT_t = outTp.tile([128, QBW], F32R, tag="outT")
                        outT_tiles.append(outT_t)
                        pend.append((pv_ps, sum_t, outT_t))
                        if len(pend) > 2:
                            emit_normalize()
                        if prev_proj is not None:
                            emit_proj_slot(prev_proj[0], prev_proj[1], h)
                    while pend:
                        emit_normalize()
                    prev_proj = (qb, outT_tiles)
                for qs in range(QBW // 128):
                    emit_proj_slot(prev_proj[0], prev_proj[1], qs)

    nc.compile()
    return nc


def _host_prep(x, wq, wk, wv, wo, freqs_cos, freqs_sin):
    x = np.asarray(x, np.float32)
    wq = np.asarray(wq, np.float32)
    wk = np.asarray(wk, np.float32)
    wv = np.asarray(wv, np.float32)
    wo = np.asarray(wo, np.float32)
    cos = np.asarray(freqs_cos, np.float32)
    sin = np.asarray(freqs_sin, np.float32)

    scale = 1.0 / np.sqrt(np.float32(HD))
    perm = np.concatenate([np.arange(0, HD, 2), np.arange(1, HD, 2)])
    wq_p = (wq.reshape(N_HEADS, HD, DIM)[:, perm, :]).reshape(DIM, DIM) * scale
    wk_p = (wk.reshape(N_KV, HD, DIM)[:, perm, :]).reshape(N_KV * HD, DIM)

    # x tiled: xT_tiled[sb, p, k, s] = x[0, sb*SBW+s, k*128+p]
    xs = x.reshape(S, DIM)
    xT_tiled = np.ascontiguousarray(
        xs.reshape(NSB, SBW, KCH, 128).transpose(0, 3, 2, 1))

    def wtile(wmat_rows):  # [rows<=128, DIM] -> [128, KCH, rows]
        return np.ascontiguousarray(
            wmat_rows.T.reshape(KCH, 128, wmat_rows.shape[0]).transpose(1, 0, 2))

    cos2 = np.ascontiguousarray(np.concatenate([cos.T, cos.T], 0))
    sin2 = np.ascontiguousarray(np.concatenate([sin.T, sin.T], 0))
    G = (np.arange(128)[:, None] <= (np.arange(896)[None, :] - 384)).astype(np.float32)
    G = np.ascontiguousarray(G)
    onesc = np.ones((128, 1), np.float32)
    onesr = np.ones((1, 128), np.float32)
    sgnv = np.concatenate([-np.ones((64, 1), np.float32),
                           np.ones((64, 1), np.float32)])
    ident = np.eye(128, dtype=np.float32)
    identn = np.eye(128, dtype=np.float32) * np.float32(-1e5)
    H = (np.arange(128)[:, None] > (np.arange(896)[None, :] - 384)).astype(np.float32)
    H = np.ascontiguousarray(H)

    in_maps = []
    for c in range(NCORES):
        wq_c = wq_p[c * FEAT:(c + 1) * FEAT]
        wqT_tiled = np.stack([wtile(wq_c[h * HD:(h + 1) * HD])
                              for h in range(HPC)])
        woc = wo[:, c * FEAT:(c + 1) * FEAT].T  # [FEAT, DIM]
        wo_tiled = np.ascontiguousarray(woc.reshape(HPC, 128, DIM))
        in_maps.append({
            "xT": xT_tiled,
            "wqT": wqT_tiled,
            "wkT": wtile(wk_p[c * HD:(c + 1) * HD]),
            "wvT": wtile(wv[c * HD:(c + 1) * HD]),
            "woT": wo_tiled,
            "cos2": cos2,
            "sin2": sin2,
            "gmask": G,
            "onesc": onesc,
            "onesr": onesr,
            "sgn": sgnv,
            "ident": ident,
            "identn": identn,
            "hmask": H,
        })
    return in_maps


def kernel(x, wq, wk, wv, wo, freqs_cos, freqs_sin, _trace=False):
    if "nc" not in _CACHE:
        _CACHE["nc"] = _build()
    nc = _CACHE["nc"]
    in_maps = _host_prep(x, wq, wk, wv, wo, freqs_cos, freqs_sin)
    res = run_bass_kernel_spmd(nc, in_maps, core_ids=list(range(NCORES)),
                               trace=_trace)
    _CACHE["last_result"] = res
    total = np.zeros((S, DIM), np.float64)
    for c in range(NCORES):
        total += res.results[c]["out"]
    return total.astype(np.float32).reshape(1, S, DIM)


# revision 20
# speedup vs baseline: 1.0173x; 1.0173x over previous
"""TRN2 Bass kernel for GQA attention (nn_Attention_19533511262498).

Tensor-parallel over heads across 8 NeuronCores: core c owns q-heads
[4c, 4c+4) and kv-head c (wq/wk/wv sharded on the head dim, wo on its
input dim). Each core computes a partial [S, DIM] output; the host sums
the 8 partials.

All matmuls run in float32r (tf32-like, ~11 mantissa bits, full PE rate
at moving-dim >= 256). Everything is kept in transposed ([feature, seq])
layouts; V is transposed on-chip via PE identity-matmuls. Softmax skips
the max-subtraction (scores are bounded by construction) and the 1/l
normalization is applied to the attention output right before the wo
projection, broadcast across partitions with a rank-1 ones matmul and
pipelined one head behind the score loop so the PE never stalls on it.

RoPE trick: wq/wk rows are permuted on the host (even dims first, odd
dims second within each head) so the rotation becomes pure elementwise
ops plus a half-tile partition swap done by SBUF-to-SBUF DMA; the
permutation cancels in q.k dot products.

Inputs are pre-tiled on the host into [128, chunk, free] layouts so
every HBM DMA moves long contiguous per-partition runs.
"""

import ml_dtypes
import numpy as np

import concourse.bacc as bacc
import concourse.tile as tile
from concourse import mybir
from concourse.bass import ts, ds
from concourse import bass_isa
from concourse.bass_utils import run_bass_kernel_spmd

F32 = mybir.dt.float32
F32R = mybir.dt.float32r
BF16 = mybir.dt.bfloat16

# problem geometry (hardcoded per contest rules)
S = 2048
DIM = 4096
HD = 128
N_HEADS = 32
N_KV = 8
NCORES = 8
HPC = N_HEADS // NCORES       # 4 q heads per core
FEAT = HPC * HD               # 512 per-core attention feature width

SBW = 256                     # QKV projection s-block width
NSB = S // SBW                # 8
KCH = DIM // 128              # 32 contraction chunks
QBW = 512                     # attention q-block width
NQB = S // QBW                # 4
NSC = S // 128                # 16 kv chunks
OBW = 512                     # output-dim block width
NOB = DIM // OBW              # 8
NRT = HPC + 1                 # rope targets per s-block (4 q heads + k)

_CACHE = {}


def _build():
    nc = bacc.Bacc("TRN2", target_bir_lowering=False, debug=False,
                   num_devices=NCORES)

    xT = nc.dram_tensor("xT", [NSB, 128, KCH, SBW], F32R, kind="ExternalInput").ap()
    wqT = nc.dram_tensor("wqT", [HPC, 128, KCH, HD], F32R, kind="ExternalInput").ap()
    wkT = nc.dram_tensor("wkT", [128, KCH, HD], F32R, kind="ExternalInput").ap()
    wvT = nc.dram_tensor("wvT", [128, KCH, HD], F32R, kind="ExternalInput").ap()
    woT = nc.dram_tensor("woT", [HPC, 128, DIM], BF16, kind="ExternalInput").ap()
    cos2 = nc.dram_tensor("cos2", [128, S], F32, kind="ExternalInput").ap()
    sin2 = nc.dram_tensor("sin2", [128, S], F32, kind="ExternalInput").ap()
    gmask = nc.dram_tensor("gmask", [128, 896], F32, kind="ExternalInput").ap()
    onesc = nc.dram_tensor("onesc", [128, 1], F32R, kind="ExternalInput").ap()
    onesr = nc.dram_tensor("onesr", [1, 128], F32R, kind="ExternalInput").ap()
    sgn = nc.dram_tensor("sgn", [128, 1], F32, kind="ExternalInput").ap()
    ident = nc.dram_tensor("ident", [128, 128], F32, kind="ExternalInput").ap()
    identn = nc.dram_tensor("identn", [128, 128], BF16, kind="ExternalInput").ap()
    hmask = nc.dram_tensor("hmask", [128, 896], BF16, kind="ExternalInput").ap()
    out_d = nc.dram_tensor("out", [S, DIM], F32, kind="ExternalOutput").ap()

    with tile.TileContext(nc) as tc:
        with (
            tc.tile_pool(name="dram", bufs=1, space="DRAM") as dpool,
            tc.tile_pool(name="res", bufs=1) as res,
        ):
            qt_ds = [dpool.tile([HPC, 128, SBW], BF16, tag=f"qt{sb}", name=f"qt_d{sb}")
                     for sb in range(NSB)]

            kt_t = res.tile([128, S], BF16, tag="kt")
            vt_t = res.tile([128, S], F32, tag="vt")
            onesc_t = res.tile([128, 1], F32R, tag="onesc")
            onesr_t = res.tile([1, 128], F32R, tag="onesr")
            sgn_t = res.tile([128, 1], F32, tag="sgn")
            nc.sync.dma_start(out=onesc_t, in_=onesc)
            nc.sync.dma_start(out=onesr_t, in_=onesr)
            nc.sync.dma_start(out=sgn_t, in_=sgn)

            # ---------------- Phase 1: QKV projections + RoPE ----------------
            with (
                tc.tile_pool(name="wq", bufs=1) as wqp,
                tc.tile_pool(name="wkv", bufs=1) as wkvp,
                tc.tile_pool(name="xt", bufs=4) as xtp,
                tc.tile_pool(name="trig", bufs=2) as trigp,
                tc.tile_pool(name="rope", bufs=6) as ropep,
                tc.tile_pool(name="stage1", bufs=2) as st1p,
                tc.tile_pool(name="qkvps", bufs=3, space="PSUM") as qkvps,
            ):
                # sync-FIFO order matters: wk/wv + first x block before the
                # 8MB of wq so the k/v matmuls can start at ~15us.
                wk_t = wkvp.tile([128, KCH, HD], F32R, tag="wk")
                nc.sync.dma_start(out=wk_t, in_=wkT)
                KH = KCH // 2

                def load_xt(sb):
                    a = xtp.tile([128, KH, SBW], F32R, tag="xt", name=f"xta{sb}")
                    nc.sync.dma_start(out=a, in_=xT[sb, :, 0:KH])
                    b = xtp.tile([128, KH, SBW], F32R, tag="xt", name=f"xtb{sb}")
                    nc.sync.dma_start(out=b, in_=xT[sb, :, KH:KCH])
                    return (a, b)

                xt_tiles = {0: load_xt(0)}
                wq_hs = [wqp.tile([128, KCH, HD], F32R, tag=f"wq{h}", name=f"wq{h}")
                         for h in range(HPC)]
                nc.sync.dma_start(out=wq_hs[0], in_=wqT[0])
                wv_t = wkvp.tile([128, KCH, HD], F32R, tag="wv")
                nc.sync.dma_start(out=wv_t, in_=wvT)
                for h in range(1, HPC):
                    nc.sync.dma_start(out=wq_hs[h], in_=wqT[h])

                for sb in range(NSB):
                    if sb not in xt_tiles:
                        xt_tiles[sb] = load_xt(sb)
                    xt_a, xt_b = xt_tiles[sb]
                    c_sl = trigp.tile([128, SBW], F32, tag="cos")
                    nc.scalar.dma_start(out=c_sl, in_=cos2[:, ts(sb, SBW)])
                    s_sl = trigp.tile([128, SBW], F32, tag="sin")
                    nc.scalar.dma_start(out=s_sl, in_=sin2[:, ts(sb, SBW)])
                    qst = st1p.tile([128, HPC, SBW], BF16, tag="qst")
                    # k, v first so matmuls start before the larger wq loads land
                    for ob in [HPC, 0, HPC + 1, 1, 2, 3]:
                        ps = qkvps.tile([128, SBW], F32, tag="ps")
                        for k in range(KCH):
                            if ob < HPC:
                                lhs = wq_hs[ob][:, k, :]
                            elif ob == HPC:
                                lhs = wk_t[:, k, :]
                            else:
                                lhs = wv_t[:, k, :]
                            rhs = xt_a[:, k, :] if k < KH else xt_b[:, k - KH, :]
                            nc.tensor.matmul(ps, lhs, rhs,
                                             start=(k == 0), stop=(k == KCH - 1))
                        if ob <= HPC:
                            # RoPE: rot = (swap_halves(x*sin) * sgn) + x*cos
                            m1 = ropep.tile([128, SBW], F32, tag="m1")
                            m2 = ropep.tile([128, SBW], F32, tag="m2")
                            w = ropep.tile([128, SBW], F32, tag="w")
                            nc.vector.tensor_mul(m1, ps, c_sl)
                            nc.vector.tensor_mul(m2, ps, s_sl)
                            nc.sync.dma_start(out=w[0:64], in_=m2[64:128])
                            nc.sync.dma_start(out=w[64:128], in_=m2[0:64])
                            dst = qst[:, ob] if ob < HPC else kt_t[:, ts(sb, SBW)]
                            nc.vector.scalar_tensor_tensor(
                                dst, w, sgn_t, m1,
                                op0=mybir.AluOpType.mult, op1=mybir.AluOpType.add)
                        else:
                            nc.vector.tensor_copy(vt_t[:, ts(sb, SBW)], ps)
                    nc.scalar.dma_start(
                        out=qt_ds[sb].rearrange("h p s -> p h s"), in_=qst)

            # ---------------- Phase 2: attention + output projection --------
            with (
                tc.tile_pool(name="wo", bufs=1) as wop,
                tc.tile_pool(name="vres", bufs=1) as vresp,
                tc.tile_pool(name="consts2", bufs=1) as c2p,
                tc.tile_pool(name="qt", bufs=4) as qtp,
                tc.tile_pool(name="exp", bufs=6) as expp,
                tc.tile_pool(name="outT", bufs=8) as outTp,
                tc.tile_pool(name="rc", bufs=4) as rcp,
                tc.tile_pool(name="small", bufs=4) as smallp,
                tc.tile_pool(name="stage2", bufs=3) as st2p,
                tc.tile_pool(name="sum", bufs=3) as sump,
                tc.tile_pool(name="bc", bufs=4) as bcp,
                tc.tile_pool(name="scps", bufs=2, space="PSUM") as scps,
                tc.tile_pool(name="pvps", bufs=3, space="PSUM") as pvps,
                tc.tile_pool(name="prps", bufs=3, space="PSUM") as prps,
            ):
                id_t = c2p.tile([128, 128], F32, tag="id")
                nc.sync.dma_start(out=id_t, in_=ident)
                idn_t = c2p.tile([128, 128], BF16, tag="idn")
                nc.sync.dma_start(out=idn_t, in_=identn)
                hm_t = c2p.tile([128, 896], BF16, tag="hm")
                nc.sync.dma_start(out=hm_t, in_=hmask)
                # wo on the scalar ring so phase-2 qt loads aren't queued
                # behind 8MB on the sync FIFO
                wo_hs = []
                for h in range(HPC):
                    wo_h = wop.tile([128, DIM], BF16, tag=f"wo{h}")
                    nc.scalar.dma_start(out=wo_h, in_=woT[h])
                    wo_hs.append(wo_h)
                # on-chip V transpose: v[s, hd] tiles from vt[hd, s]
                v_t = vresp.tile([128, NSC, HD], BF16, tag="v")
                for sc in range(NSC):
                    tr_ps = scps.tile([128, HD], F32, tag="sc")
                    nc.tensor.transpose(tr_ps, vt_t[:, ts(sc, 128)], id_t)
                    nc.vector.tensor_copy(v_t[:, sc, :], tr_ps)

                # Normalizers run 2 heads behind the score loop (chunked so
                # the first 128-q slice is ready ~1.5us after the sums); the
                # whole projection runs one q-block behind the attention.
                pend = []  # list of (pv_ps, sum_t, outT_t)

                def emit_normalize():
                    pv_ps, sum_t, outT_t = pend.pop(0)
                    for cs in range(QBW // 128):
                        c = ts(cs, 128)
                        bc_c = bcp.tile([128, 128], F32, tag="bc")
                        nc.gpsimd.partition_all_reduce(
                            bc_c, sum_t[:, c], channels=128,
                            reduce_op=bass_isa.ReduceOp.add)
                        rc_c = rcp.tile([128, 128], F32, tag="rc")
                        nc.vector.reciprocal_approx_fast(out=rc_c, in_=bc_c)
                        nc.vector.tensor_mul(outT_t[:, c], pv_ps[:, c], rc_c)

                def emit_proj_slot(pqb, tiles, qs):
                    o_st = st2p.tile([128, DIM], F32, tag="ost")
                    for ob in range(NOB):
                        p_ps = prps.tile([128, OBW], F32, tag="pr")
                        for h in range(HPC):
                            nc.tensor.matmul(p_ps,
                                             tiles[h][:, ts(qs, 128)],
                                             wo_hs[h][:, ts(ob, OBW)],
                                             start=(h == 0), stop=(h == HPC - 1))
                        nc.vector.tensor_copy(o_st[:, ts(ob, OBW)], p_ps)
                    nc.scalar.dma_start(
                        out=out_d[ds(pqb * QBW + qs * 128, 128), :], in_=o_st)

                prev_proj = None  # (qb, outT_tiles)
                for qb in range(NQB):
                    outT_tiles = []
                    for h in range(HPC):
                        qt_t = qtp.tile([128, 2, SBW], BF16, tag="qt")
                        nc.sync.dma_start(out=qt_t[:, 0, :],
                                          in_=qt_ds[2 * qb][h].rearrange("p s -> p s"))
                        nc.sync.dma_start(out=qt_t[:, 1, :],
                                          in_=qt_ds[2 * qb + 1][h])
                        qt_v = qt_t.rearrange("p b s -> p (b s)")
                        pv_ps = pvps.tile([128, QBW], F32, tag="pv")
                        sum_t = sump.tile([128, QBW], F32, tag="sum")
                        nsc = 4 * (qb + 1)
                        for sc in range(nsc):
                            s_ps = scps.tile([128, QBW], F32, tag="sc")
                            diag = sc >= 4 * qb
                            nc.tensor.matmul(s_ps, kt_t[:, ts(sc, 128)], qt_v,
                                             start=True, stop=not diag)
                            if diag:
                                # causal mask: accumulate -1e5 * step pattern
                                t = sc - 4 * qb
                                nc.tensor.matmul(
                                    s_ps, idn_t, hm_t[:, ds(384 - 128 * t, QBW)],
                                    start=False, stop=True)
                            e_t = expp.tile([128, QBW], BF16, tag="exp")
                            nc.scalar.activation(e_t, s_ps,
                                                 mybir.ActivationFunctionType.Exp)
                            if sc == 0:
                                nc.vector.tensor_copy(sum_t, e_t)
                            else:
                                nc.vector.tensor_add(sum_t, sum_t, e_t)
                            nc.tensor.matmul(pv_ps, v_t[:, sc, :], e_t,
                                             start=(sc == 0), stop=(sc == nsc - 1))
                        outT_t = outTp.tile([128, QBW], BF16, tag="outT")
                        outT_tiles.append(outT_t)
                        pend.append((pv_ps, sum_t, outT_t))
                        if len(pend) > 2:
                            emit_normalize()
                        if prev_proj is not None:
                            emit_proj_slot(prev_proj[0], prev_proj[1], h)
                    while pend:
                        emit_normalize()
                    prev_proj = (qb, outT_tiles)
                for qs in range(QBW // 128):
                    emit_proj_slot(prev_proj[0], prev_proj[1], qs)

    nc.compile()
    return nc


def _host_prep(x, wq, wk, wv, wo, freqs_cos, freqs_sin):
    x = np.asarray(x, np.float32)
    wq = np.asarray(wq, np.float32)
    wk = np.asarray(wk, np.float32)
    wv = np.asarray(wv, np.float32)
    wo = np.asarray(wo, np.float32)
    cos = np.asarray(freqs_cos, np.float32)
    sin = np.asarray(freqs_sin, np.float32)

    scale = 1.0 / np.sqrt(np.float32(HD))
    perm = np.concatenate([np.arange(0, HD, 2), np.arange(1, HD, 2)])
    wq_p = (wq.reshape(N_HEADS, HD, DIM)[:, perm, :]).reshape(DIM, DIM) * scale
    wk_p = (wk.reshape(N_KV, HD, DIM)[:, perm, :]).reshape(N_KV * HD, DIM)

    # x tiled: xT_tiled[sb, p, k, s] = x[0, sb*SBW+s, k*128+p]
    xs = x.reshape(S, DIM)
    xT_tiled = np.ascontiguousarray(
        xs.reshape(NSB, SBW, KCH, 128).transpose(0, 3, 2, 1))

    def wtile(wmat_rows):  # [rows<=128, DIM] -> [128, KCH, rows]
        return np.ascontiguousarray(
            wmat_rows.T.reshape(KCH, 128, wmat_rows.shape[0]).transpose(1, 0, 2))

    cos2 = np.ascontiguousarray(np.concatenate([cos.T, cos.T], 0))
    sin2 = np.ascontiguousarray(np.concatenate([sin.T, sin.T], 0))
    G = (np.arange(128)[:, None] <= (np.arange(896)[None, :] - 384)).astype(np.float32)
    G = np.ascontiguousarray(G)
    onesc = np.ones((128, 1), np.float32)
    onesr = np.ones((1, 128), np.float32)
    sgnv = np.concatenate([-np.ones((64, 1), np.float32),
                           np.ones((64, 1), np.float32)])
    ident = np.eye(128, dtype=np.float32)
    identn = (np.eye(128, dtype=np.float32) * np.float32(-1e5)).astype(ml_dtypes.bfloat16)
    H = (np.arange(128)[:, None] > (np.arange(896)[None, :] - 384)).astype(ml_dtypes.bfloat16)
    H = np.ascontiguousarray(H)

    in_maps = []
    for c in range(NCORES):
        wq_c = wq_p[c * FEAT:(c + 1) * FEAT]
        wqT_tiled = np.stack([wtile(wq_c[h * HD:(h + 1) * HD])
                              for h in range(HPC)])
        woc = wo[:, c * FEAT:(c + 1) * FEAT].T  # [FEAT, DIM]
        wo_tiled = np.ascontiguousarray(
            woc.reshape(HPC, 128, DIM).astype(ml_dtypes.bfloat16))
        in_maps.append({
            "xT": xT_tiled,
            "wqT": wqT_tiled,
            "wkT": wtile(wk_p[c * HD:(c + 1) * HD]),
            "wvT": wtile(wv[c * HD:(c + 1) * HD]),
            "woT": wo_tiled,
            "cos2": cos2,
            "sin2": sin2,
            "gmask": G,
            "onesc": onesc,
            "onesr": onesr,
            "sgn": sgnv,
            "ident": ident,
            "identn": identn,
            "hmask": H,
        })
    return in_maps


def kernel(x, wq, wk, wv, wo, freqs_cos, freqs_sin, _trace=False):
    if "nc" not in _CACHE:
        _CACHE["nc"] = _build()
    nc = _CACHE["nc"]
    in_maps = _host_prep(x, wq, wk, wv, wo, freqs_cos, freqs_sin)
    res = run_bass_kernel_spmd(nc, in_maps, core_ids=list(range(NCORES)),
                               trace=_trace)
    _CACHE["last_result"] = res
    total = np.zeros((S, DIM), np.float64)
    for c in range(NCORES):
        total += res.results[c]["out"]
    return total.astype(np.float32).reshape(1, S, DIM)


# revision 21
# speedup vs baseline: 1.0846x; 1.0662x over previous
"""TRN2 Bass kernel for GQA attention (nn_Attention_19533511262498).

Tensor-parallel over heads across 8 NeuronCores: core c owns q-heads
[4c, 4c+4) and kv-head c (wq/wk/wv sharded on the head dim, wo on its
input dim). Each core computes a partial [S, DIM] output; the host sums
the 8 partials.

All matmuls run in float32r (tf32-like, ~11 mantissa bits, full PE rate
at moving-dim >= 256). Everything is kept in transposed ([feature, seq])
layouts; V is transposed on-chip via PE identity-matmuls. Softmax skips
the max-subtraction (scores are bounded by construction) and the 1/l
normalization is applied to the attention output right before the wo
projection, broadcast across partitions with a rank-1 ones matmul and
pipelined one head behind the score loop so the PE never stalls on it.

RoPE trick: wq/wk rows are permuted on the host (even dims first, odd
dims second within each head) so the rotation becomes pure elementwise
ops plus a half-tile partition swap done by SBUF-to-SBUF DMA; the
permutation cancels in q.k dot products.

Inputs are pre-tiled on the host into [128, chunk, free] layouts so
every HBM DMA moves long contiguous per-partition runs.
"""

import ml_dtypes
import numpy as np

import concourse.bacc as bacc
import concourse.tile as tile
from concourse import mybir
from concourse.bass import ts, ds
from concourse import bass_isa
from concourse.bass_utils import run_bass_kernel_spmd

F32 = mybir.dt.float32
F32R = mybir.dt.float32r
BF16 = mybir.dt.bfloat16

# problem geometry (hardcoded per contest rules)
S = 2048
DIM = 4096
HD = 128
N_HEADS = 32
N_KV = 8
NCORES = 8
HPC = N_HEADS // NCORES       # 4 q heads per core
FEAT = HPC * HD               # 512 per-core attention feature width

SBW = 256                     # QKV projection s-block width
NSB = S // SBW                # 8
KCH = DIM // 128              # 32 contraction chunks
QBW = 512                     # attention q-block width
NQB = S // QBW                # 4
NSC = S // 128                # 16 kv chunks
OBW = 512                     # output-dim block width
NOB = DIM // OBW              # 8
NRT = HPC + 1                 # rope targets per s-block (4 q heads + k)

_CACHE = {}


def _build():
    nc = bacc.Bacc("TRN2", target_bir_lowering=False, debug=False,
                   num_devices=NCORES)

    xT = nc.dram_tensor("xT", [NSB, 128, KCH, SBW], F32R, kind="ExternalInput").ap()
    wqT = nc.dram_tensor("wqT", [HPC, 128, KCH, HD], F32R, kind="ExternalInput").ap()
    wkT = nc.dram_tensor("wkT", [128, KCH, HD], F32R, kind="ExternalInput").ap()
    wvT = nc.dram_tensor("wvT", [128, KCH, HD], F32R, kind="ExternalInput").ap()
    woT = nc.dram_tensor("woT", [HPC, 128, DIM], F32R, kind="ExternalInput").ap()
    cos2 = nc.dram_tensor("cos2", [128, S], F32, kind="ExternalInput").ap()
    sin2 = nc.dram_tensor("sin2", [128, S], F32, kind="ExternalInput").ap()
    gmask = nc.dram_tensor("gmask", [128, 896], F32, kind="ExternalInput").ap()
    onesc = nc.dram_tensor("onesc", [128, 1], F32R, kind="ExternalInput").ap()
    onesr = nc.dram_tensor("onesr", [1, 128], F32R, kind="ExternalInput").ap()
    sgn = nc.dram_tensor("sgn", [128, 1], F32, kind="ExternalInput").ap()
    ident = nc.dram_tensor("ident", [128, 128], F32, kind="ExternalInput").ap()
    identn = nc.dram_tensor("identn", [128, 128], F32R, kind="ExternalInput").ap()
    hmask = nc.dram_tensor("hmask", [128, 896], F32R, kind="ExternalInput").ap()
    out_d = nc.dram_tensor("out", [S, DIM], F32, kind="ExternalOutput").ap()

    with tile.TileContext(nc) as tc:
        with (
            tc.tile_pool(name="dram", bufs=1, space="DRAM") as dpool,
            tc.tile_pool(name="res", bufs=1) as res,
        ):
            qt_ds = [dpool.tile([HPC, 128, SBW], F32R, tag=f"qt{sb}", name=f"qt_d{sb}")
                     for sb in range(NSB)]

            kt_t = res.tile([128, S], F32R, tag="kt")
            vt_t = res.tile([128, S], F32, tag="vt")
            onesc_t = res.tile([128, 1], F32R, tag="onesc")
            onesr_t = res.tile([1, 128], F32R, tag="onesr")
            sgn_t = res.tile([128, 1], F32, tag="sgn")
            nc.sync.dma_start(out=onesc_t, in_=onesc)
            nc.sync.dma_start(out=onesr_t, in_=onesr)
            nc.sync.dma_start(out=sgn_t, in_=sgn)

            # ---------------- Phase 1: QKV projections + RoPE ----------------
            with (
                tc.tile_pool(name="wq", bufs=1) as wqp,
                tc.tile_pool(name="wkv", bufs=1) as wkvp,
                tc.tile_pool(name="xt", bufs=4) as xtp,
                tc.tile_pool(name="trig", bufs=2) as trigp,
                tc.tile_pool(name="rope", bufs=6) as ropep,
                tc.tile_pool(name="stage1", bufs=2) as st1p,
                tc.tile_pool(name="qkvps", bufs=3, space="PSUM") as qkvps,
            ):
                # sync-FIFO order matters: wk/wv + first x block before the
                # 8MB of wq so the k/v matmuls can start at ~15us.
                wk_t = wkvp.tile([128, KCH, HD], F32R, tag="wk")
                nc.sync.dma_start(out=wk_t, in_=wkT)
                KH = KCH // 2

                def load_xt(sb):
                    a = xtp.tile([128, KH, SBW], F32R, tag="xt", name=f"xta{sb}")
                    nc.sync.dma_start(out=a, in_=xT[sb, :, 0:KH])
                    b = xtp.tile([128, KH, SBW], F32R, tag="xt", name=f"xtb{sb}")
                    nc.sync.dma_start(out=b, in_=xT[sb, :, KH:KCH])
                    return (a, b)

                xt_tiles = {0: load_xt(0)}
                wq_hs = [wqp.tile([128, KCH, HD], F32R, tag=f"wq{h}", name=f"wq{h}")
                         for h in range(HPC)]
                nc.sync.dma_start(out=wq_hs[0], in_=wqT[0])
                wv_t = wkvp.tile([128, KCH, HD], F32R, tag="wv")
                nc.sync.dma_start(out=wv_t, in_=wvT)
                for h in range(1, HPC):
                    nc.sync.dma_start(out=wq_hs[h], in_=wqT[h])

                for sb in range(NSB):
                    if sb not in xt_tiles:
                        xt_tiles[sb] = load_xt(sb)
                    xt_a, xt_b = xt_tiles[sb]
                    c_sl = trigp.tile([128, SBW], F32, tag="cos")
                    nc.scalar.dma_start(out=c_sl, in_=cos2[:, ts(sb, SBW)])
                    s_sl = trigp.tile([128, SBW], F32, tag="sin")
                    nc.scalar.dma_start(out=s_sl, in_=sin2[:, ts(sb, SBW)])
                    qst = st1p.tile([128, HPC, SBW], F32R, tag="qst")
                    # k, v first so matmuls start before the larger wq loads land
                    for ob in [HPC, 0, HPC + 1, 1, 2, 3]:
                        ps = qkvps.tile([128, SBW], F32, tag="ps")
                        for k in range(KCH):
                            if ob < HPC:
                                lhs = wq_hs[ob][:, k, :]
                            elif ob == HPC:
                                lhs = wk_t[:, k, :]
                            else:
                                lhs = wv_t[:, k, :]
                            rhs = xt_a[:, k, :] if k < KH else xt_b[:, k - KH, :]
                            nc.tensor.matmul(ps, lhs, rhs,
                                             start=(k == 0), stop=(k == KCH - 1))
                        if ob <= HPC:
                            # RoPE: rot = (swap_halves(x*sin) * sgn) + x*cos
                            m1 = ropep.tile([128, SBW], F32, tag="m1")
                            m2 = ropep.tile([128, SBW], F32, tag="m2")
                            w = ropep.tile([128, SBW], F32, tag="w")
                            nc.vector.tensor_mul(m1, ps, c_sl)
                            nc.vector.tensor_mul(m2, ps, s_sl)
                            nc.sync.dma_start(out=w[0:64], in_=m2[64:128])
                            nc.sync.dma_start(out=w[64:128], in_=m2[0:64])
                            dst = qst[:, ob] if ob < HPC else kt_t[:, ts(sb, SBW)]
                            nc.vector.scalar_tensor_tensor(
                                dst, w, sgn_t, m1,
                                op0=mybir.AluOpType.mult, op1=mybir.AluOpType.add)
                        else:
                            nc.vector.tensor_copy(vt_t[:, ts(sb, SBW)], ps)
                    nc.scalar.dma_start(
                        out=qt_ds[sb].rearrange("h p s -> p h s"), in_=qst)

            # ---------------- Phase 2: attention + output projection --------
            with (
                tc.tile_pool(name="wo", bufs=1) as wop,
                tc.tile_pool(name="vres", bufs=1) as vresp,
                tc.tile_pool(name="consts2", bufs=1) as c2p,
                tc.tile_pool(name="qt", bufs=4) as qtp,
                tc.tile_pool(name="exp", bufs=6) as expp,
                tc.tile_pool(name="outT", bufs=8) as outTp,
                tc.tile_pool(name="rc", bufs=4) as rcp,
                tc.tile_pool(name="small", bufs=4) as smallp,
                tc.tile_pool(name="stage2", bufs=3) as st2p,
                tc.tile_pool(name="sum", bufs=3) as sump,
                tc.tile_pool(name="bc", bufs=4) as bcp,
                tc.tile_pool(name="scps", bufs=2, space="PSUM") as scps,
                tc.tile_pool(name="pvps", bufs=3, space="PSUM") as pvps,
                tc.tile_pool(name="prps", bufs=3, space="PSUM") as prps,
            ):
                id_t = c2p.tile([128, 128], F32, tag="id")
                nc.sync.dma_start(out=id_t, in_=ident)
                idn_t = c2p.tile([128, 128], F32R, tag="idn")
                nc.sync.dma_start(out=idn_t, in_=identn)
                hm_t = c2p.tile([128, 896], F32R, tag="hm")
                nc.sync.dma_start(out=hm_t, in_=hmask)
                # wo on the scalar ring so phase-2 qt loads aren't queued
                # behind 8MB on the sync FIFO
                wo_hs = []
                for h in range(HPC):
                    wo_h = wop.tile([128, DIM], F32R, tag=f"wo{h}")
                    nc.scalar.dma_start(out=wo_h, in_=woT[h])
                    wo_hs.append(wo_h)
                # on-chip V transpose: v[s, hd] tiles from vt[hd, s]
                v_t = vresp.tile([128, NSC, HD], F32R, tag="v")
                for sc in range(NSC):
                    tr_ps = scps.tile([128, HD], F32, tag="sc")
                    nc.tensor.transpose(tr_ps, vt_t[:, ts(sc, 128)], id_t)
                    nc.vector.tensor_copy(v_t[:, sc, :], tr_ps)

                # Normalizers run 2 heads behind the score loop (chunked so
                # the first 128-q slice is ready ~1.5us after the sums); the
                # whole projection runs one q-block behind the attention.
                pend = []  # list of (pv_ps, sum_t, outT_t)

                def emit_normalize():
                    pv_ps, sum_t, outT_t = pend.pop(0)
                    for cs in range(QBW // 128):
                        c = ts(cs, 128)
                        bc_c = bcp.tile([128, 128], F32, tag="bc")
                        nc.gpsimd.partition_all_reduce(
                            bc_c, sum_t[:, c], channels=128,
                            reduce_op=bass_isa.ReduceOp.add)
                        rc_c = rcp.tile([128, 128], F32, tag="rc")
                        nc.vector.reciprocal_approx_fast(out=rc_c, in_=bc_c)
                        nc.vector.tensor_mul(outT_t[:, c], pv_ps[:, c], rc_c)

                def emit_proj_slot(pqb, tiles, qs):
                    o_st = st2p.tile([128, DIM], F32, tag="ost")
                    for ob in range(NOB):
                        p_ps = prps.tile([128, OBW], F32, tag="pr")
                        for h in range(HPC):
                            nc.tensor.matmul(p_ps,
                                             tiles[h][:, ts(qs, 128)],
                                             wo_hs[h][:, ts(ob, OBW)],
                                             start=(h == 0), stop=(h == HPC - 1))
                        nc.scalar.copy(o_st[:, ts(ob, OBW)], p_ps)
                    nc.scalar.dma_start(
                        out=out_d[ds(pqb * QBW + qs * 128, 128), :], in_=o_st)

                prev_proj = None  # (qb, outT_tiles)
                for qb in range(NQB):
                    outT_tiles = []
                    for h in range(HPC):
                        qt_t = qtp.tile([128, 2, SBW], F32R, tag="qt")
                        nc.sync.dma_start(out=qt_t[:, 0, :],
                                          in_=qt_ds[2 * qb][h].rearrange("p s -> p s"))
                        nc.sync.dma_start(out=qt_t[:, 1, :],
                                          in_=qt_ds[2 * qb + 1][h])
                        qt_v = qt_t.rearrange("p b s -> p (b s)")
                        pv_ps = pvps.tile([128, QBW], F32, tag="pv")
                        sum_t = sump.tile([128, QBW], F32, tag="sum")
                        nsc = 4 * (qb + 1)
                        for sc in range(nsc):
                            s_ps = scps.tile([128, QBW], F32, tag="sc")
                            diag = sc >= 4 * qb
                            nc.tensor.matmul(s_ps, kt_t[:, ts(sc, 128)], qt_v,
                                             start=True, stop=not diag)
                            if diag:
                                # causal mask: accumulate -1e5 * step pattern
                                t = sc - 4 * qb
                                nc.tensor.matmul(
                                    s_ps, idn_t, hm_t[:, ds(384 - 128 * t, QBW)],
                                    start=False, stop=True)
                            e_t = expp.tile([128, QBW], F32R, tag="exp")
                            nc.scalar.activation(e_t, s_ps,
                                                 mybir.ActivationFunctionType.Exp)
                            e32 = e_t.bitcast(F32)
                            if sc == 0:
                                nc.vector.tensor_copy(sum_t, e32)
                            else:
                                nc.vector.tensor_add(sum_t, sum_t, e32)
                            nc.tensor.matmul(pv_ps, v_t[:, sc, :], e_t,
                                             start=(sc == 0), stop=(sc == nsc - 1))
                        outT_t = outTp.tile([128, QBW], F32R, tag="outT")
                        outT_tiles.append(outT_t)
                        pend.append((pv_ps, sum_t, outT_t))
                        if len(pend) > 2:
                            emit_normalize()
                        if prev_proj is not None:
                            emit_proj_slot(prev_proj[0], prev_proj[1], h)
                    while pend:
                        emit_normalize()
                    prev_proj = (qb, outT_tiles)
                for qs in range(QBW // 128):
                    emit_proj_slot(prev_proj[0], prev_proj[1], qs)

    nc.compile()
    return nc


def _host_prep(x, wq, wk, wv, wo, freqs_cos, freqs_sin):
    x = np.asarray(x, np.float32)
    wq = np.asarray(wq, np.float32)
    wk = np.asarray(wk, np.float32)
    wv = np.asarray(wv, np.float32)
    wo = np.asarray(wo, np.float32)
    cos = np.asarray(freqs_cos, np.float32)
    sin = np.asarray(freqs_sin, np.float32)

    scale = 1.0 / np.sqrt(np.float32(HD))
    perm = np.concatenate([np.arange(0, HD, 2), np.arange(1, HD, 2)])
    wq_p = (wq.reshape(N_HEADS, HD, DIM)[:, perm, :]).reshape(DIM, DIM) * scale
    wk_p = (wk.reshape(N_KV, HD, DIM)[:, perm, :]).reshape(N_KV * HD, DIM)

    # x tiled: xT_tiled[sb, p, k, s] = x[0, sb*SBW+s, k*128+p]
    xs = x.reshape(S, DIM)
    xT_tiled = np.ascontiguousarray(
        xs.reshape(NSB, SBW, KCH, 128).transpose(0, 3, 2, 1))

    def wtile(wmat_rows):  # [rows<=128, DIM] -> [128, KCH, rows]
        return np.ascontiguousarray(
            wmat_rows.T.reshape(KCH, 128, wmat_rows.shape[0]).transpose(1, 0, 2))

    cos2 = np.ascontiguousarray(np.concatenate([cos.T, cos.T], 0))
    sin2 = np.ascontiguousarray(np.concatenate([sin.T, sin.T], 0))
    G = (np.arange(128)[:, None] <= (np.arange(896)[None, :] - 384)).astype(np.float32)
    G = np.ascontiguousarray(G)
    onesc = np.ones((128, 1), np.float32)
    onesr = np.ones((1, 128), np.float32)
    sgnv = np.concatenate([-np.ones((64, 1), np.float32),
                           np.ones((64, 1), np.float32)])
    ident = np.eye(128, dtype=np.float32)
    identn = np.eye(128, dtype=np.float32) * np.float32(-1e5)
    H = (np.arange(128)[:, None] > (np.arange(896)[None, :] - 384)).astype(np.float32)
    H = np.ascontiguousarray(H)

    in_maps = []
    for c in range(NCORES):
        wq_c = wq_p[c * FEAT:(c + 1) * FEAT]
        wqT_tiled = np.stack([wtile(wq_c[h * HD:(h + 1) * HD])
                              for h in range(HPC)])
        woc = wo[:, c * FEAT:(c + 1) * FEAT].T  # [FEAT, DIM]
        wo_tiled = np.ascontiguousarray(woc.reshape(HPC, 128, DIM))
        in_maps.append({
            "xT": xT_tiled,
            "wqT": wqT_tiled,
            "wkT": wtile(wk_p[c * HD:(c + 1) * HD]),
            "wvT": wtile(wv[c * HD:(c + 1) * HD]),
            "woT": wo_tiled,
            "cos2": cos2,
            "sin2": sin2,
            "gmask": G,
            "onesc": onesc,
            "onesr": onesr,
            "sgn": sgnv,
            "ident": ident,
            "identn": identn,
            "hmask": H,
        })
    return in_maps


def kernel(x, wq, wk, wv, wo, freqs_cos, freqs_sin, _trace=False):
    if "nc" not in _CACHE:
        _CACHE["nc"] = _build()
    nc = _CACHE["nc"]
    in_maps = _host_prep(x, wq, wk, wv, wo, freqs_cos, freqs_sin)
    res = run_bass_kernel_spmd(nc, in_maps, core_ids=list(range(NCORES)),
                               trace=_trace)
    _CACHE["last_result"] = res
    total = np.zeros((S, DIM), np.float64)
    for c in range(NCORES):
        total += res.results[c]["out"]
    return total.astype(np.float32).reshape(1, S, DIM)
